# revision 15
# baseline (speedup 1.0000x reference)
"""Trainium2 Bass kernel: cubic B-spline upsampling x2 of a (2,3,96,96,96) volume.

Math: the reference op (recursive IIR prefilter along each spatial axis, then
an 8-tap stride-2 transposed conv along each axis) is linear and separable.
The whole per-axis operator is a dense 192x96 matrix M (built exactly on the
host in float64).  out = M (x) M (x) M applied along z, y, x.

Device strategy (8 NeuronCores, SPMD, no collectives), v2:
  24 slices = 6 (b,c) volumes x 4 z'-slices of 48 rows; 3 per core, arranged
  so tasks t0,t1 share one volume (slot0) and t2 uses a second (slot1) --
  input DMA is 2 volume loads per core instead of 3, z unpadded (96 rows).
  Three data-stationary matmul stages (stationary = data tile, moving =
  spline matrix), so no transposes are needed:
    A: per 8-x group: lhsT = vol[z96, (x,y128)]  rhs=MzT-slices -> (y, z')
       slot0 runs t0+t1 fused (N=96, one LDWEIGHTS pass over the volume)
    B: per z': lhsT = L1[y96, x96]   rhs = MT (96,192) -> (x, y')
    C: per 128-chunk of (z'y'): lhsT = L2f[x96, chunk] rhs = MT -> (chunk, x')
  All PSUM evacuation copies are FD=768 (2 PSUM banks) and are split across
  DVE and ACT by a weighted round-robin (both engines read PSUM at 1 f32/cyc;
  this is the critical resource).  Output staged [128, 12, 192] bf16 and
  DMA'd with 4608B-contiguous per-partition runs (>=512B avoids the SDMA
  read-modify-write penalty that made 384B-run writes ~2x slower).  C-groups
  of task t are partly held back and ride along the next phase so the
  out-DMA stream never starves.  Compute bf16 (PSUM fp32); output written
  bf16, upcast on host (rel err ~5.3e-3 vs the reference).
"""

import math
import os
import sys

import numpy as np

for _p in ("/opt/trn_rl_repo",):
    if _p not in sys.path and os.path.isdir(_p):
        sys.path.insert(0, _p)

import ml_dtypes  # noqa: E402

BF16 = ml_dtypes.bfloat16

POLE = math.sqrt(3.0) - 2.0
GAIN = (1.0 - POLE) * (1.0 - 1.0 / POLE)  # 6.0
N = 96
F = 2
NOUT = N * F  # 192
NCORES = 8
ZSLICE = 48

# per core: 3 tasks as (volume_index, zslice_index); t0,t1 share slot0's
# volume, t2 uses slot1's (cores with one distinct volume duplicate it)
TASKS = [
    [(0, 0), (0, 1), (0, 2)],
    [(1, 0), (1, 1), (0, 3)],
    [(1, 2), (1, 3), (2, 0)],
    [(2, 1), (2, 2), (2, 3)],
    [(3, 0), (3, 1), (3, 2)],
    [(4, 0), (4, 1), (3, 3)],
    [(4, 2), (4, 3), (5, 0)],
    [(5, 1), (5, 2), (5, 3)],
]


def _cubic(t):
    a = np.abs(t)
    out = (2.0 / 3.0 + (0.5 * a - 1.0) * a**2) * (a < 1)
    out = out + (-((a - 2.0) ** 3) / 6.0) * ((a >= 1) & (a < 2))
    return out


def _prefilter_mat(n):
    """96x96 matrix of the causal+anticausal cubic-spline prefilter (float64)."""
    p = POLE
    xm = np.eye(n, dtype=np.float64) * GAIN
    i = np.arange(n)
    pows = p**i + p ** (2 * n - 1 - i)
    c = np.zeros((n, n), dtype=np.float64)
    c[0] = (pows @ xm) * (p / (1.0 - p ** (2 * n))) + xm[0]
    for k in range(1, n):
        c[k] = xm[k] + p * c[k - 1]
    out = np.zeros((n, n), dtype=np.float64)
    out[n - 1] = c[n - 1] * (p / (p - 1.0))
    for k in range(n - 2, -1, -1):
        out[k] = p * (out[k + 1] - c[k])
    return out


def _upsample_mat(n, f=F):
    """2n x n matrix of the edge-padded stride-2 transposed conv (float64)."""
    k = 4 * f  # f even -> is_odd == 0
    start = 1.0 / (2 * f) - 2.0
    pts = np.arange(k, dtype=np.float64) * (1.0 / f) + start
    ker = _cubic(pts)
    npad = n + 4
    U = np.zeros((f * n, npad), dtype=np.float64)
    for o in range(f * n):
        for i in range(npad):
            s = o + (k - 1) - f * i
            if 0 <= s < k:
                U[o, i] += ker[s]
    Uc = np.zeros((f * n, n), dtype=np.float64)
    for i in range(npad):
        j = min(max(i - 2, 0), n - 1)
        Uc[:, j] += U[:, i]
    return Uc


def build_M():
    """Exact 192x96 per-axis operator (float64)."""
    return _upsample_mat(N) @ _prefilter_mat(N)


_NC_CACHE = {}


def _strip_redundant_self_waits(nc):
    """Drop sem waits that are trivially satisfied by same-engine program order.

    Tile's per-proc wait emission is not transitively minimal: a PE matmul can
    end up waiting on the PE's own semaphore (already guaranteed by in-order
    engine execution) in addition to a cross-engine wait, and the MM ISA
    struct only has one sync-wait slot (walrus: "Too many sync wait
    commands"). A wait on sem S is redundant for instruction I on engine E iff
    S is only ever updated by E and the cumulative updates to S from E before
    I already reach the wait value.
    """
    import concourse.mybir as mybir

    for fn in nc.m.functions:
        for blk in fn.blocks:
            updaters = {}  # sem id -> set of engines updating it (block-wide)
            for i in blk.instructions:
                si = i.sync_info
                if si is None:
                    continue
                for u in si.on_update or []:
                    updaters.setdefault(u.id, set()).add(i.engine)
            seen = {}  # (engine, sem id) -> cumulative update count so far
            for i in blk.instructions:
                si = i.sync_info
                if si is None:
                    continue
                if si.on_wait:
                    kept = []
                    for w in si.on_wait:
                        if (
                            w.sync_type == "semaphore"
                            and w.wait_mode == "sem-ge-imm"
                            and updaters.get(w.id) == {i.engine}
                            and seen.get((i.engine, w.id), 0) >= w.wait_value
                        ):
                            continue  # implied by program order
                        kept.append(w)
                    if len(kept) != len(si.on_wait):
                        si.on_wait[:] = kept
                for u in si.on_update or []:
                    key = (i.engine, u.id)
                    seen[key] = seen.get(key, 0) + u.update_value
            # each engine ISA struct has a single sync-wait slot: offload
            # extra waits onto same-engine nops inserted just before
            new_insts = []
            nop_n = 0
            for i in blk.instructions:
                si = i.sync_info
                if si is not None and si.on_wait and len(si.on_wait) > 1:
                    extra = list(si.on_wait[:-1])
                    si.on_wait[:] = [si.on_wait[-1]]
                    for w in extra:
                        nop = mybir.InstNoOp(
                            name=f"I-waitnop-{blk.name}-{nop_n}", ins=[], outs=[]
                        )
                        nop_n += 1
                        nop.engine = i.engine
                        nop.sync_info = mybir.SyncInfo(on_wait=[w], on_update=[])
                        new_insts.append(nop)
                new_insts.append(i)
            if nop_n:
                blk.instructions[:] = new_insts


def _hoist_input_dmas(nc, n_hoist=3):
    """Move the first input DMAs ahead of the sync engine's entry barrier.

    The Tile/BSP prologue (entry EVSEM barrier + TENSOR_LOAD) delays the
    first dma_start by several us. The leading input DMAs have no waits
    (inputs are resident at NEFF start, dst tiles untouched), so issuing
    them first starts the HBM reads during the prologue.
    """
    import concourse.mybir as mybir

    blocks = nc.m.functions[0].blocks
    body = blocks[1]
    dmas = []
    for i in body.instructions:
        if type(i).__name__ == "InstDMACopy" and i.engine == mybir.EngineType.SP:
            si = i.sync_info
            if si is not None and si.on_wait:
                break  # stop at the first gated DMA
            dmas.append(i)
            if len(dmas) >= n_hoist:
                break
    if not dmas:
        return
    dset = set(id(x) for x in dmas)
    body.instructions[:] = [i for i in body.instructions if id(i) not in dset]
    # insert into the prologue block after the leading InstCall, ahead of
    # the entry barrier: the sync engine starts immediately, so these DMAs
    # issue at t~0 while the other engines are still loading their code
    pro = blocks[0].instructions
    pos = 1 if pro and type(pro[0]).__name__ == "InstCall" else 0
    pro[:] = pro[:pos] + dmas + pro[pos:]


def build_nc():
    import concourse.bass as bass
    import concourse.mybir as mybir
    from concourse.tile import TileContext

    bf16 = mybir.dt.bfloat16
    f32 = mybir.dt.float32

    nc = bass.Bass(enable_partition_id=False)
    # inputs: 2 volume slots (z unpadded: 96 rows; y padded to 128 per
    # x-slice for 128-col FWL stationaries), spline matrices packed in one
    # [96, 336] tile: [0:96] = MzT slices for t0|t1, [96:144] = MzT for t2,
    # [144:336] = MT
    vol_ext = nc.declare_dram_parameter("vol", [2, 96, 96 * 128], bf16, isOutput=False)
    mts_ext = nc.declare_dram_parameter("mts", [96, 336], bf16, isOutput=False)
    # out rows: task t, stage-group d (12 chunks of 128 (z'y')-rows each),
    # partition-major inside: per partition 2304 elems contiguous
    out_ext = nc.declare_dram_parameter("out", [128, 3, 6, 2304], bf16, isOutput=True)
    dbg = os.environ.get("KDBG") == "1"
    if dbg:
        dbg_l1 = nc.declare_dram_parameter("dbg_l1", [96, 3, 48 * 96], bf16, isOutput=True)
        dbg_l2 = nc.declare_dram_parameter("dbg_l2", [96, 3, 9216], bf16, isOutput=True)

    with TileContext(nc) as tc:
        with (
            tc.tile_pool(name="consts", bufs=1) as consts,
            tc.tile_pool(name="vols", bufs=2) as vols_pool,
            tc.tile_pool(name="l1", bufs=2) as l1_pool,
            tc.tile_pool(name="l2", bufs=3) as l2_pool,
            tc.tile_pool(name="stage", bufs=8) as stage_pool,
            tc.tile_pool(name="pab", bufs=2, space="PSUM") as pab_pool,
            tc.tile_pool(name="pc", bufs=2, space="PSUM") as pc_pool,
        ):
            mts = consts.tile([96, 336], bf16)
            nc.sync.dma_start(out=mts[:], in_=mts_ext[:])
            # prime the scalar engine's activation table during the input
            # DMA wait (ACT_TABLE_LOAD ~1.3us otherwise lands on the first
            # real C-copy); the dummy also touches nothing downstream
            prime = consts.tile([96, 136], bf16)
            nc.gpsimd.memset(prime[:], 0.0)
            nc.scalar.copy(prime[0:1, 4:8], prime[0:1, 0:4])

            # graduated input chunks: small early ones so stage A starts as
            # soon as the first x-slices land
            def load_vol(slot, bounds):
                vol = vols_pool.tile([96, 96 * 128], bf16, name="vol")
                for ch in range(len(bounds) - 1):
                    nc.sync.dma_start(
                        out=vol[:, bounds[ch] : bounds[ch + 1]],
                        in_=vol_ext[slot, :, bounds[ch] : bounds[ch + 1]],
                    )
                return vol

            vol0 = load_vol(0, [0, 1024, 2048, 4096, 6656, 9472, 12288])
            vol1 = load_vol(1, [0, 4096, 8192, 12288])

            # ---- PSUM evacuation engine assignment ----
            # both engines read PSUM at 1 f32/cycle.  strict per-stream
            # alternation: consecutive generations of one stream go to
            # different engines so each stream gets 2-wide copy concurrency
            # (different PSUM banks, so DVE+ACT run in parallel).
            tick = {"n": 0}

            def copy_psum(stream, dst, src):
                # global strict alternation: emission order approximates
                # readiness order, so alternating engines per copy keeps both
                # PSUM read ports busy with no same-engine back-to-back pairs
                t = tick["n"]
                tick["n"] = t + 1
                if t % 2 == 0:
                    nc.vector.tensor_copy(dst, src)
                else:
                    nc.scalar.copy(dst, src)

            # ---- stage C emitter per task ----
            # 72 chunks of 128 (z'y')-rows; groups of 4 chunks (one 2-bank
            # PSUM tile); 3 groups fill one [128, 12, 192] stage tile ->
            # one out-DMA with 4608B contiguous per-partition runs.
            def make_emit_c(t, L2f):
                state = {"stage": None}

                def emit_c_group(cg):
                    if cg % 3 == 0:
                        state["stage"] = stage_pool.tile([128, 12, 192], bf16, name="stage")
                    stage = state["stage"]
                    pc = pc_pool.tile(
                        [128, 4, 192], f32, name="pc", tag="pc", padded_shape=[128, 4, 256]
                    )
                    for j in range(4):
                        ch = cg * 4 + j
                        nc.tensor.matmul(
                            pc[:, j, :],
                            lhsT=L2f[:, ch * 128 : (ch + 1) * 128],
                            rhs=mts[:, 144:336],
                            start=True,
                            stop=True,
                        )
                    copy_psum("c", stage[:, (cg % 3) * 4 : (cg % 3) * 4 + 4, :], pc[:, :, :])
                    if t == 2 and cg == 13:
                        nc.sync.dma_start(
                            out=out_ext[:, t, 4, 0:1536],
                            in_=stage[:, 0:8, :].rearrange("p c x -> p (c x)"),
                        )
                    elif t == 2 and cg == 14:
                        nc.sync.dma_start(
                            out=out_ext[:, t, 4, 1536:2304],
                            in_=stage[:, 8:12, :].rearrange("p c x -> p (c x)"),
                        )
                    elif t == 2 and cg == 16:
                        # fine tail: first half of the last group goes out as
                        # soon as it is ready, shortening the final drain
                        nc.sync.dma_start(
                            out=out_ext[:, t, 5, 0:1152],
                            in_=stage[:, 0:6, :].rearrange("p c x -> p (c x)"),
                        )
                    elif t == 2 and cg == 17:
                        nc.sync.dma_start(
                            out=out_ext[:, t, 5, 1152:2304],
                            in_=stage[:, 6:12, :].rearrange("p c x -> p (c x)"),
                        )
                    elif cg % 3 == 2:
                        d = cg // 3
                        nc.sync.dma_start(
                            out=out_ext[:, t, d, :],
                            in_=stage[:].rearrange("p c x -> p (c x)"),
                        )

                return emit_c_group

            # ---- stage A ----
            # slot0: fused t0+t1 (N=96); slot1: t2 alone (N=48).
            # pa partitions = y (96 real + 32 junk from the y-pad cols); only
            # [0:96] is copied out, so the junk never propagates.
            def emit_a0(vol, L1a):
                for g in range(12):  # groups of 8 x-slices -> 2 PSUM banks
                    pool = pab_pool if g % 2 == 0 else pc_pool
                    pa = pool.tile(
                        [128, 8, 96], f32, name="pa0",
                        tag="pab" if g % 2 == 0 else "pc",
                        padded_shape=[128, 8, 128],
                    )
                    for j in range(8):
                        x = g * 8 + j
                        nc.tensor.matmul(
                            pa[:, j, :],
                            lhsT=vol[:, x * 128 : (x + 1) * 128],
                            rhs=mts[:, 0:96],
                            start=True,
                            stop=True,
                        )
                    copy_psum(
                        "a0",
                        L1a[:, :, :, g * 8 : (g + 1) * 8],
                        pa[0:96, :, :].rearrange("p j (t z) -> p t z j", t=2),
                    )

            def make_a1_thunks(vol, L1b):
                def mk(g):
                    def thunk():
                        pa = pab_pool.tile(
                            [128, 16, 48], f32, name="pa1", tag="pab",
                            padded_shape=[128, 16, 64],
                        )
                        for j in range(16):
                            x = g * 16 + j
                            nc.tensor.matmul(
                                pa[:, j, :],
                                lhsT=vol[:, x * 128 : (x + 1) * 128],
                                rhs=mts[:, 96:144],
                                start=True,
                                stop=True,
                            )
                        copy_psum(
                            "a1",
                            L1b[:, :, g * 16 : (g + 1) * 16],
                            pa[0:96, :, :].rearrange("p j z -> p z j"),
                        )
                    return thunk

                return [mk(g) for g in range(6)]

            # ---- unified B/C scheduler ----
            # After A0, the three tasks' B-streams round-robin (B2 joins once
            # slot1's stage A finishes) and each task's C-groups are emitted
            # as soon as their L2 rows are safely ahead.  One continuous
            # phase: no pipeline-refill bubbles at task boundaries, PE never
            # idles long enough for HAM to re-throttle, and the copy engines
            # always have 2+ independent streams to alternate across.
            def emit_a1_group(vol, L1b, g):
                pa = pab_pool.tile(
                    [128, 16, 48], f32, name="pa1", tag="pab",
                    padded_shape=[128, 16, 64],
                )
                for j in range(16):
                    x = g * 16 + j
                    nc.tensor.matmul(
                        pa[:, j, :],
                        lhsT=vol[:, x * 128 : (x + 1) * 128],
                        rhs=mts[:, 96:144],
                        start=True,
                        stop=True,
                    )
                copy_psum(
                    "a1",
                    L1b[:, :, g * 16 : (g + 1) * 16],
                    pa[0:96, :, :].rearrange("p j z -> p z j"),
                )

            class BC:
                def __init__(self, t, l1_slice):
                    self.t = t
                    self.l1_slice = l1_slice
                    self.L2 = l2_pool.tile([96, 48, 192], bf16, name="l2")
                    self.L2f = self.L2[:].rearrange("p a b -> p (a b)")
                    self.emit_c = make_emit_c(t, self.L2f)
                    self.b_done = 0
                    self.c_done = 0

                def step_b(self):
                    pb = pab_pool.tile(
                        [128, 4, 192], f32, name="pb", tag="pab",
                        padded_shape=[128, 4, 256],
                    )
                    for jj in range(4):
                        zp = self.b_done * 4 + jj
                        nc.tensor.matmul(
                            pb[0:96, jj, :],
                            lhsT=self.l1_slice(zp),
                            rhs=mts[:, 144:336],
                            start=True,
                            stop=True,
                        )
                    copy_psum(
                        "b",
                        self.L2[:, self.b_done * 4 : self.b_done * 4 + 4, :],
                        pb[0:96, :, :],
                    )
                    self.b_done += 1

                def pump_c(self, max_groups=2):
                    # one-B-group slack while B is in flight: a C-group at
                    # the exact readiness boundary would head-of-line block
                    # the PE queue waiting on the just-emitted B copy
                    rows = self.b_done * 768 - (768 if self.b_done < 12 else 0)
                    n = 0
                    while (
                        self.c_done < 18
                        and n < max_groups
                        and (self.c_done + 1) * 512 <= rows
                    ):
                        self.emit_c(self.c_done)
                        self.c_done += 1
                        n += 1

            # ---- schedule ----
            L1a = l1_pool.tile([96, 2, 48, 96], bf16, name="l1a")
            emit_a0(vol0, L1a)
            if dbg:
                nc.sync.dma_start(out=dbg_l1[:, 0:2, :], in_=L1a[:].rearrange("p t z x -> p t (z x)"))

            L1b = l1_pool.tile([96, 48, 96], bf16, name="l1b")
            streams = [
                BC(0, lambda zp: L1a[:, 0, zp, :]),
                BC(1, lambda zp: L1a[:, 1, zp, :]),
            ]
            s2 = BC(2, lambda zp: L1b[:, zp, :])
            a1_next = 0
            total_b = 0
            rr = 0
            while True:
                alive = [s for s in streams if s.b_done < 12]
                if not alive and a1_next >= 6:
                    break
                if alive:
                    s = alive[rr % len(alive)]
                    rr += 1
                    s.step_b()
                    total_b += 1
                if a1_next < 6:
                    emit_a1_group(vol1, L1b, a1_next)
                    a1_next += 1
                    if a1_next == 6:
                        streams.append(s2)
                for s in streams:
                    s.pump_c(2)
            if dbg:
                nc.sync.dma_start(out=dbg_l1[:, 2, :], in_=L1b[:].rearrange("p z x -> p (z x)"))
            while any(s.c_done < 18 for s in streams):
                for s in streams:
                    s.pump_c(3)
            if dbg:
                for s in streams:
                    nc.sync.dma_start(out=dbg_l2[:, s.t, :], in_=s.L2f)

    if os.environ.get("KNOPASS") != "1":
        _strip_redundant_self_waits(nc)
        _hoist_input_dmas(nc)
    return nc


def make_in_maps(volume, M):
    MT = np.ascontiguousarray(M.T).astype(BF16)  # (96, 192)
    in_maps = []
    for core in range(NCORES):
        tasks = TASKS[core]
        slot_vols = [tasks[0][0], tasks[2][0]]
        vols = np.zeros((2, 96, 96, 128), dtype=BF16)
        for s, v in enumerate(slot_vols):
            b, c = divmod(v, 3)
            vt = np.transpose(volume[b, c], (0, 2, 1))  # (z, x, y)
            vols[s, :, :, :96] = vt.astype(BF16)
        mts = np.zeros((96, 336), dtype=BF16)
        for t in range(2):
            sl = tasks[t][1]
            mts[:, t * 48 : (t + 1) * 48] = MT[:, sl * ZSLICE : (sl + 1) * ZSLICE]
        sl = tasks[2][1]
        mts[:, 96:144] = MT[:, sl * ZSLICE : (sl + 1) * ZSLICE]
        mts[:, 144:336] = MT
        in_maps.append({"vol": vols.reshape(2, 96, 96 * 128), "mts": mts})
    return in_maps


def gather_out(results):
    out = np.zeros((2, 3, 192, 192, 192), dtype=np.float32)
    for core in range(NCORES):
        o = np.asarray(results[core]["out"], dtype=np.float32)
        # [128, 3, 6, 2304] -> (t, d, cc, p, x) -> task rows (z'y') x x'
        o = o.reshape(128, 3, 6, 12, 192).transpose(1, 2, 3, 0, 4)
        o = o.reshape(3, 9216, 192).reshape(3, 48, 192, 192)
        for t in range(3):
            v, s = TASKS[core][t]
            b, c = divmod(v, 3)
            out[b, c, s * ZSLICE : (s + 1) * ZSLICE] = o[t]
    return out


def run(volume, trace=False):
    """Returns (output, exec_time_ns_or_None)."""
    import concourse.bass_utils as bu
    from concourse.bass_utils import run_bass_kernel_spmd

    if trace:
        # avoid the S3 artifact upload in the axon trace path
        bu.upload_artifacts = lambda tmpdir: str(tmpdir)

    volume = np.asarray(volume, dtype=np.float32)
    M = build_M()
    in_maps = make_in_maps(volume, M)
    if "nc" not in _NC_CACHE:
        _NC_CACHE["nc"] = build_nc()
    nc = _NC_CACHE["nc"]
    res = run_bass_kernel_spmd(
        nc, in_maps, core_ids=list(range(NCORES)), trace=trace
    )
    out = gather_out(res.results)
    return out, getattr(res, "exec_time_ns", None)


def kernel(volume):
    out, _ = run(volume, trace=False)
    return out


# revision 16
# speedup vs baseline: 1.0687x; 1.0687x over previous
"""Trainium2 Bass kernel: cubic B-spline upsampling x2 of a (2,3,96,96,96) volume.

Math: the reference op (recursive IIR prefilter along each spatial axis, then
an 8-tap stride-2 transposed conv along each axis) is linear and separable.
The whole per-axis operator is a dense 192x96 matrix M (built exactly on the
host in float64).  out = M (x) M (x) M applied along z, y, x.

Device strategy (8 NeuronCores, SPMD, no collectives), v2:
  24 slices = 6 (b,c) volumes x 4 z'-slices of 48 rows; 3 per core, arranged
  so tasks t0,t1 share one volume (slot0) and t2 uses a second (slot1) --
  input DMA is 2 volume loads per core instead of 3, z unpadded (96 rows).
  Three data-stationary matmul stages (stationary = data tile, moving =
  spline matrix), so no transposes are needed:
    A: per 8-x group: lhsT = vol[z96, (x,y128)]  rhs=MzT-slices -> (y, z')
       slot0 runs t0+t1 fused (N=96, one LDWEIGHTS pass over the volume)
    B: per z': lhsT = L1[y96, x96]   rhs = MT (96,192) -> (x, y')
    C: per 128-chunk of (z'y'): lhsT = L2f[x96, chunk] rhs = MT -> (chunk, x')
  All PSUM evacuation copies are FD=768 (2 PSUM banks) and are split across
  DVE and ACT by a weighted round-robin (both engines read PSUM at 1 f32/cyc;
  this is the critical resource).  Output staged [128, 12, 192] bf16 and
  DMA'd with 4608B-contiguous per-partition runs (>=512B avoids the SDMA
  read-modify-write penalty that made 384B-run writes ~2x slower).  C-groups
  of task t are partly held back and ride along the next phase so the
  out-DMA stream never starves.  Compute bf16 (PSUM fp32); output written
  bf16, upcast on host (rel err ~5.3e-3 vs the reference).
"""

import math
import os
import sys

import numpy as np

for _p in ("/opt/trn_rl_repo",):
    if _p not in sys.path and os.path.isdir(_p):
        sys.path.insert(0, _p)

import ml_dtypes  # noqa: E402

BF16 = ml_dtypes.bfloat16

POLE = math.sqrt(3.0) - 2.0
GAIN = (1.0 - POLE) * (1.0 - 1.0 / POLE)  # 6.0
N = 96
F = 2
NOUT = N * F  # 192
NCORES = 8
ZSLICE = 48

# per core: 3 tasks as (volume_index, zslice_index); t0,t1 share slot0's
# volume, t2 uses slot1's (cores with one distinct volume duplicate it)
TASKS = [
    [(0, 0), (0, 1), (0, 2)],
    [(1, 0), (1, 1), (0, 3)],
    [(1, 2), (1, 3), (2, 0)],
    [(2, 1), (2, 2), (2, 3)],
    [(3, 0), (3, 1), (3, 2)],
    [(4, 0), (4, 1), (3, 3)],
    [(4, 2), (4, 3), (5, 0)],
    [(5, 1), (5, 2), (5, 3)],
]


def _cubic(t):
    a = np.abs(t)
    out = (2.0 / 3.0 + (0.5 * a - 1.0) * a**2) * (a < 1)
    out = out + (-((a - 2.0) ** 3) / 6.0) * ((a >= 1) & (a < 2))
    return out


def _prefilter_mat(n):
    """96x96 matrix of the causal+anticausal cubic-spline prefilter (float64)."""
    p = POLE
    xm = np.eye(n, dtype=np.float64) * GAIN
    i = np.arange(n)
    pows = p**i + p ** (2 * n - 1 - i)
    c = np.zeros((n, n), dtype=np.float64)
    c[0] = (pows @ xm) * (p / (1.0 - p ** (2 * n))) + xm[0]
    for k in range(1, n):
        c[k] = xm[k] + p * c[k - 1]
    out = np.zeros((n, n), dtype=np.float64)
    out[n - 1] = c[n - 1] * (p / (p - 1.0))
    for k in range(n - 2, -1, -1):
        out[k] = p * (out[k + 1] - c[k])
    return out


def _upsample_mat(n, f=F):
    """2n x n matrix of the edge-padded stride-2 transposed conv (float64)."""
    k = 4 * f  # f even -> is_odd == 0
    start = 1.0 / (2 * f) - 2.0
    pts = np.arange(k, dtype=np.float64) * (1.0 / f) + start
    ker = _cubic(pts)
    npad = n + 4
    U = np.zeros((f * n, npad), dtype=np.float64)
    for o in range(f * n):
        for i in range(npad):
            s = o + (k - 1) - f * i
            if 0 <= s < k:
                U[o, i] += ker[s]
    Uc = np.zeros((f * n, n), dtype=np.float64)
    for i in range(npad):
        j = min(max(i - 2, 0), n - 1)
        Uc[:, j] += U[:, i]
    return Uc


def build_M():
    """Exact 192x96 per-axis operator (float64)."""
    return _upsample_mat(N) @ _prefilter_mat(N)


_NC_CACHE = {}


def _strip_redundant_self_waits(nc):
    """Drop sem waits that are trivially satisfied by same-engine program order.

    Tile's per-proc wait emission is not transitively minimal: a PE matmul can
    end up waiting on the PE's own semaphore (already guaranteed by in-order
    engine execution) in addition to a cross-engine wait, and the MM ISA
    struct only has one sync-wait slot (walrus: "Too many sync wait
    commands"). A wait on sem S is redundant for instruction I on engine E iff
    S is only ever updated by E and the cumulative updates to S from E before
    I already reach the wait value.
    """
    import concourse.mybir as mybir

    for fn in nc.m.functions:
        for blk in fn.blocks:
            updaters = {}  # sem id -> set of engines updating it (block-wide)
            for i in blk.instructions:
                si = i.sync_info
                if si is None:
                    continue
                for u in si.on_update or []:
                    updaters.setdefault(u.id, set()).add(i.engine)
            seen = {}  # (engine, sem id) -> cumulative update count so far
            for i in blk.instructions:
                si = i.sync_info
                if si is None:
                    continue
                if si.on_wait:
                    kept = []
                    for w in si.on_wait:
                        if (
                            w.sync_type == "semaphore"
                            and w.wait_mode == "sem-ge-imm"
                            and updaters.get(w.id) == {i.engine}
                            and seen.get((i.engine, w.id), 0) >= w.wait_value
                        ):
                            continue  # implied by program order
                        kept.append(w)
                    if len(kept) != len(si.on_wait):
                        si.on_wait[:] = kept
                for u in si.on_update or []:
                    key = (i.engine, u.id)
                    seen[key] = seen.get(key, 0) + u.update_value
            # each engine ISA struct has a single sync-wait slot: offload
            # extra waits onto same-engine nops inserted just before
            new_insts = []
            nop_n = 0
            for i in blk.instructions:
                si = i.sync_info
                if si is not None and si.on_wait and len(si.on_wait) > 1:
                    extra = list(si.on_wait[:-1])
                    si.on_wait[:] = [si.on_wait[-1]]
                    for w in extra:
                        nop = mybir.InstNoOp(
                            name=f"I-waitnop-{blk.name}-{nop_n}", ins=[], outs=[]
                        )
                        nop_n += 1
                        nop.engine = i.engine
                        nop.sync_info = mybir.SyncInfo(on_wait=[w], on_update=[])
                        new_insts.append(nop)
                new_insts.append(i)
            if nop_n:
                blk.instructions[:] = new_insts


def _hoist_input_dmas(nc, n_hoist=3):
    """Move the first input DMAs ahead of the sync engine's entry barrier.

    The Tile/BSP prologue (entry EVSEM barrier + TENSOR_LOAD) delays the
    first dma_start by several us. The leading input DMAs have no waits
    (inputs are resident at NEFF start, dst tiles untouched), so issuing
    them first starts the HBM reads during the prologue.
    """
    import concourse.mybir as mybir

    blocks = nc.m.functions[0].blocks
    body = blocks[1]
    dmas = []
    for i in body.instructions:
        if type(i).__name__ == "InstDMACopy" and i.engine == mybir.EngineType.SP:
            si = i.sync_info
            if si is not None and si.on_wait:
                break  # stop at the first gated DMA
            dmas.append(i)
            if len(dmas) >= n_hoist:
                break
    if not dmas:
        return
    dset = set(id(x) for x in dmas)
    body.instructions[:] = [i for i in body.instructions if id(i) not in dset]
    # insert into the prologue block after the leading InstCall, ahead of
    # the entry barrier: the sync engine starts immediately, so these DMAs
    # issue at t~0 while the other engines are still loading their code
    pro = blocks[0].instructions
    pos = 1 if pro and type(pro[0]).__name__ == "InstCall" else 0
    pro[:] = pro[:pos] + dmas + pro[pos:]


def build_nc():
    import concourse.bass as bass
    import concourse.mybir as mybir
    from concourse.tile import TileContext

    bf16 = mybir.dt.bfloat16
    f32 = mybir.dt.float32

    nc = bass.Bass(enable_partition_id=False)
    # inputs: 2 volume slots (z unpadded: 96 rows; y padded to 128 per
    # x-slice for 128-col FWL stationaries), spline matrices packed in one
    # [96, 336] tile: [0:96] = MzT slices for t0|t1, [96:144] = MzT for t2,
    # [144:336] = MT
    vol_ext = nc.declare_dram_parameter("vol", [2, 96, 96 * 128], bf16, isOutput=False)
    mts_ext = nc.declare_dram_parameter("mts", [96, 336], bf16, isOutput=False)
    # out rows: task t, stage-group d (12 chunks of 128 (z'y')-rows each),
    # partition-major inside: per partition 2304 elems contiguous
    out_ext = nc.declare_dram_parameter("out", [128, 3, 6, 2304], bf16, isOutput=True)
    dbg = os.environ.get("KDBG") == "1"
    if dbg:
        dbg_l1 = nc.declare_dram_parameter("dbg_l1", [96, 3, 48 * 96], bf16, isOutput=True)
        dbg_l2 = nc.declare_dram_parameter("dbg_l2", [96, 3, 9216], bf16, isOutput=True)

    with TileContext(nc) as tc:
        with (
            tc.tile_pool(name="consts", bufs=1) as consts,
            tc.tile_pool(name="vols", bufs=2) as vols_pool,
            tc.tile_pool(name="l1", bufs=2) as l1_pool,
            tc.tile_pool(name="l2", bufs=3) as l2_pool,
            tc.tile_pool(name="stage", bufs=8) as stage_pool,
            tc.tile_pool(name="pab", bufs=2, space="PSUM") as pab_pool,
            tc.tile_pool(name="pc", bufs=2, space="PSUM") as pc_pool,
        ):
            mts = consts.tile([96, 336], bf16)
            nc.sync.dma_start(out=mts[:], in_=mts_ext[:])
            # prime the scalar engine's activation table during the input
            # DMA wait (ACT_TABLE_LOAD ~1.3us otherwise lands on the first
            # real C-copy); the dummy also touches nothing downstream
            prime = consts.tile([96, 136], bf16)
            nc.gpsimd.memset(prime[:], 0.0)
            nc.scalar.copy(prime[0:1, 4:8], prime[0:1, 0:4])

            # graduated input chunks: small early ones so stage A starts as
            # soon as the first x-slices land
            def load_vol(slot, bounds):
                vol = vols_pool.tile([96, 96 * 128], bf16, name="vol")
                for ch in range(len(bounds) - 1):
                    nc.sync.dma_start(
                        out=vol[:, bounds[ch] : bounds[ch + 1]],
                        in_=vol_ext[slot, :, bounds[ch] : bounds[ch + 1]],
                    )
                return vol

            vol0 = load_vol(0, [0, 1024, 2048, 4096, 6656, 9472, 12288])
            vol1 = load_vol(1, [0, 4096, 8192, 12288])

            # ---- PSUM evacuation engine assignment ----
            # both engines read PSUM at 1 f32/cycle.  strict per-stream
            # alternation: consecutive generations of one stream go to
            # different engines so each stream gets 2-wide copy concurrency
            # (different PSUM banks, so DVE+ACT run in parallel).
            tick = {"n": 0}

            def copy_psum(stream, dst, src):
                # global strict alternation: emission order approximates
                # readiness order, so alternating engines per copy keeps both
                # PSUM read ports busy with no same-engine back-to-back pairs
                t = tick["n"]
                tick["n"] = t + 1
                if t % 2 == 0:
                    nc.vector.tensor_copy(dst, src)
                else:
                    nc.scalar.copy(dst, src)

            # ---- stage C emitter per task ----
            # 72 chunks of 128 (z'y')-rows; groups of 4 chunks (one 2-bank
            # PSUM tile); 3 groups fill one [128, 12, 192] stage tile ->
            # one out-DMA with 4608B contiguous per-partition runs.
            def make_emit_c(t, L2f):
                state = {"stage": None}

                def emit_c_group(cg):
                    if cg % 3 == 0:
                        state["stage"] = stage_pool.tile([128, 12, 192], bf16, name="stage")
                    stage = state["stage"]
                    pc = pc_pool.tile(
                        [128, 4, 192], f32, name="pc", tag="pc", padded_shape=[128, 4, 256]
                    )
                    for j in range(4):
                        ch = cg * 4 + j
                        nc.tensor.matmul(
                            pc[:, j, :],
                            lhsT=L2f[:, ch * 128 : (ch + 1) * 128],
                            rhs=mts[:, 144:336],
                            start=True,
                            stop=True,
                        )
                    copy_psum("c", stage[:, (cg % 3) * 4 : (cg % 3) * 4 + 4, :], pc[:, :, :])
                    if t == 2 and cg == 13:
                        nc.sync.dma_start(
                            out=out_ext[:, t, 4, 0:1536],
                            in_=stage[:, 0:8, :].rearrange("p c x -> p (c x)"),
                        )
                    elif t == 2 and cg == 14:
                        nc.sync.dma_start(
                            out=out_ext[:, t, 4, 1536:2304],
                            in_=stage[:, 8:12, :].rearrange("p c x -> p (c x)"),
                        )
                    elif t == 2 and cg == 16:
                        # fine tail: first half of the last group goes out as
                        # soon as it is ready, shortening the final drain
                        nc.sync.dma_start(
                            out=out_ext[:, t, 5, 0:1152],
                            in_=stage[:, 0:6, :].rearrange("p c x -> p (c x)"),
                        )
                    elif t == 2 and cg == 17:
                        nc.sync.dma_start(
                            out=out_ext[:, t, 5, 1152:2304],
                            in_=stage[:, 6:12, :].rearrange("p c x -> p (c x)"),
                        )
                    elif cg % 3 == 2:
                        d = cg // 3
                        nc.sync.dma_start(
                            out=out_ext[:, t, d, :],
                            in_=stage[:].rearrange("p c x -> p (c x)"),
                        )

                return emit_c_group

            # ---- stage A ----
            # slot0: fused t0+t1 (N=96); slot1: t2 alone (N=48).
            # pa partitions = y (96 real + 32 junk from the y-pad cols); only
            # [0:96] is copied out, so the junk never propagates.
            def emit_a0(vol, L1a):
                for g in range(12):  # groups of 8 x-slices -> 2 PSUM banks
                    pool = pab_pool if g % 2 == 0 else pc_pool
                    pa = pool.tile(
                        [128, 8, 96], f32, name="pa0",
                        tag="pab" if g % 2 == 0 else "pc",
                        padded_shape=[128, 8, 128],
                    )
                    for j in range(8):
                        x = g * 8 + j
                        nc.tensor.matmul(
                            pa[:, j, :],
                            lhsT=vol[:, x * 128 : (x + 1) * 128],
                            rhs=mts[:, 0:96],
                            start=True,
                            stop=True,
                        )
                    copy_psum(
                        "a0",
                        L1a[:, :, :, g * 8 : (g + 1) * 8],
                        pa[0:96, :, :].rearrange("p j (t z) -> p t z j", t=2),
                    )

            def make_a1_thunks(vol, L1b):
                def mk(g):
                    def thunk():
                        pa = pab_pool.tile(
                            [128, 16, 48], f32, name="pa1", tag="pab",
                            padded_shape=[128, 16, 64],
                        )
                        for j in range(16):
                            x = g * 16 + j
                            nc.tensor.matmul(
                                pa[:, j, :],
                                lhsT=vol[:, x * 128 : (x + 1) * 128],
                                rhs=mts[:, 96:144],
                                start=True,
                                stop=True,
                            )
                        copy_psum(
                            "a1",
                            L1b[:, :, g * 16 : (g + 1) * 16],
                            pa[0:96, :, :].rearrange("p j z -> p z j"),
                        )
                    return thunk

                return [mk(g) for g in range(6)]

            # ---- unified B/C scheduler ----
            # After A0, the three tasks' B-streams round-robin (B2 joins once
            # slot1's stage A finishes) and each task's C-groups are emitted
            # as soon as their L2 rows are safely ahead.  One continuous
            # phase: no pipeline-refill bubbles at task boundaries, PE never
            # idles long enough for HAM to re-throttle, and the copy engines
            # always have 2+ independent streams to alternate across.
            def emit_a1_group(vol, L1b, g):
                pa = pab_pool.tile(
                    [128, 16, 48], f32, name="pa1", tag="pab",
                    padded_shape=[128, 16, 64],
                )
                for j in range(16):
                    x = g * 16 + j
                    nc.tensor.matmul(
                        pa[:, j, :],
                        lhsT=vol[:, x * 128 : (x + 1) * 128],
                        rhs=mts[:, 96:144],
                        start=True,
                        stop=True,
                    )
                copy_psum(
                    "a1",
                    L1b[:, :, g * 16 : (g + 1) * 16],
                    pa[0:96, :, :].rearrange("p j z -> p z j"),
                )

            class BC:
                def __init__(self, t, l1_slice):
                    self.t = t
                    self.l1_slice = l1_slice
                    self.L2 = l2_pool.tile([96, 48, 192], bf16, name="l2")
                    self.L2f = self.L2[:].rearrange("p a b -> p (a b)")
                    self.emit_c = make_emit_c(t, self.L2f)
                    self.b_done = 0
                    self.c_done = 0

                def step_b(self):
                    pb = pab_pool.tile(
                        [128, 4, 192], f32, name="pb", tag="pab",
                        padded_shape=[128, 4, 256],
                    )
                    for jj in range(4):
                        zp = self.b_done * 4 + jj
                        nc.tensor.matmul(
                            pb[0:96, jj, :],
                            lhsT=self.l1_slice(zp),
                            rhs=mts[:, 144:336],
                            start=True,
                            stop=True,
                        )
                    copy_psum(
                        "b",
                        self.L2[:, self.b_done * 4 : self.b_done * 4 + 4, :],
                        pb[0:96, :, :],
                    )
                    self.b_done += 1

                def pump_c(self, max_groups=2):
                    # one-B-group slack while B is in flight: a C-group at
                    # the exact readiness boundary would head-of-line block
                    # the PE queue waiting on the just-emitted B copy
                    rows = self.b_done * 768 - (768 if self.b_done < 12 else 0)
                    n = 0
                    while (
                        self.c_done < 18
                        and n < max_groups
                        and (self.c_done + 1) * 512 <= rows
                    ):
                        self.emit_c(self.c_done)
                        self.c_done += 1
                        n += 1

            # ---- schedule ----
            L1a = l1_pool.tile([96, 2, 48, 96], bf16, name="l1a")
            emit_a0(vol0, L1a)
            if dbg:
                nc.sync.dma_start(out=dbg_l1[:, 0:2, :], in_=L1a[:].rearrange("p t z x -> p t (z x)"))

            L1b = l1_pool.tile([96, 48, 96], bf16, name="l1b")
            streams = [
                BC(0, lambda zp: L1a[:, 0, zp, :]),
                BC(1, lambda zp: L1a[:, 1, zp, :]),
            ]
            s2 = BC(2, lambda zp: L1b[:, zp, :])
            a1_next = 0
            total_b = 0
            rr = 0
            while True:
                alive = [s for s in streams if s.b_done < 12]
                if not alive and a1_next >= 6:
                    break
                if alive:
                    s = alive[rr % len(alive)]
                    rr += 1
                    s.step_b()
                    total_b += 1
                if a1_next < 6 and total_b % 2 == 0:
                    emit_a1_group(vol1, L1b, a1_next)
                    a1_next += 1
                    if a1_next == 6:
                        streams.append(s2)
                for s in streams:
                    s.pump_c(2)
            if dbg:
                nc.sync.dma_start(out=dbg_l1[:, 2, :], in_=L1b[:].rearrange("p z x -> p (z x)"))
            while any(s.c_done < 18 for s in streams):
                for s in streams:
                    s.pump_c(3)
            if dbg:
                for s in streams:
                    nc.sync.dma_start(out=dbg_l2[:, s.t, :], in_=s.L2f)

    if os.environ.get("KNOPASS") != "1":
        _strip_redundant_self_waits(nc)
        _hoist_input_dmas(nc)
    return nc


def make_in_maps(volume, M):
    MT = np.ascontiguousarray(M.T).astype(BF16)  # (96, 192)
    in_maps = []
    for core in range(NCORES):
        tasks = TASKS[core]
        slot_vols = [tasks[0][0], tasks[2][0]]
        vols = np.zeros((2, 96, 96, 128), dtype=BF16)
        for s, v in enumerate(slot_vols):
            b, c = divmod(v, 3)
            vt = np.transpose(volume[b, c], (0, 2, 1))  # (z, x, y)
            vols[s, :, :, :96] = vt.astype(BF16)
        mts = np.zeros((96, 336), dtype=BF16)
        for t in range(2):
            sl = tasks[t][1]
            mts[:, t * 48 : (t + 1) * 48] = MT[:, sl * ZSLICE : (sl + 1) * ZSLICE]
        sl = tasks[2][1]
        mts[:, 96:144] = MT[:, sl * ZSLICE : (sl + 1) * ZSLICE]
        mts[:, 144:336] = MT
        in_maps.append({"vol": vols.reshape(2, 96, 96 * 128), "mts": mts})
    return in_maps


def gather_out(results):
    out = np.zeros((2, 3, 192, 192, 192), dtype=np.float32)
    for core in range(NCORES):
        o = np.asarray(results[core]["out"], dtype=np.float32)
        # [128, 3, 6, 2304] -> (t, d, cc, p, x) -> task rows (z'y') x x'
        o = o.reshape(128, 3, 6, 12, 192).transpose(1, 2, 3, 0, 4)
        o = o.reshape(3, 9216, 192).reshape(3, 48, 192, 192)
        for t in range(3):
            v, s = TASKS[core][t]
            b, c = divmod(v, 3)
            out[b, c, s * ZSLICE : (s + 1) * ZSLICE] = o[t]
    return out


def run(volume, trace=False):
    """Returns (output, exec_time_ns_or_None)."""
    import concourse.bass_utils as bu
    from concourse.bass_utils import run_bass_kernel_spmd

    if trace:
        # avoid the S3 artifact upload in the axon trace path
        bu.upload_artifacts = lambda tmpdir: str(tmpdir)

    volume = np.asarray(volume, dtype=np.float32)
    M = build_M()
    in_maps = make_in_maps(volume, M)
    if "nc" not in _NC_CACHE:
        _NC_CACHE["nc"] = build_nc()
    nc = _NC_CACHE["nc"]
    res = run_bass_kernel_spmd(
        nc, in_maps, core_ids=list(range(NCORES)), trace=trace
    )
    out = gather_out(res.results)
    return out, getattr(res, "exec_time_ns", None)


def kernel(volume):
    out, _ = run(volume, trace=False)
    return out


# revision 17
# speedup vs baseline: 1.0774x; 1.0081x over previous
"""Trainium2 Bass kernel: cubic B-spline upsampling x2 of a (2,3,96,96,96) volume.

Math: the reference op (recursive IIR prefilter along each spatial axis, then
an 8-tap stride-2 transposed conv along each axis) is linear and separable.
The whole per-axis operator is a dense 192x96 matrix M (built exactly on the
host in float64).  out = M (x) M (x) M applied along z, y, x.

Device strategy (8 NeuronCores, SPMD, no collectives), v2:
  24 slices = 6 (b,c) volumes x 4 z'-slices of 48 rows; 3 per core, arranged
  so tasks t0,t1 share one volume (slot0) and t2 uses a second (slot1) --
  input DMA is 2 volume loads per core instead of 3, z unpadded (96 rows).
  Three data-stationary matmul stages (stationary = data tile, moving =
  spline matrix), so no transposes are needed:
    A: per 8-x group: lhsT = vol[z96, (x,y128)]  rhs=MzT-slices -> (y, z')
       slot0 runs t0+t1 fused (N=96, one LDWEIGHTS pass over the volume)
    B: per z': lhsT = L1[y96, x96]   rhs = MT (96,192) -> (x, y')
    C: per 128-chunk of (z'y'): lhsT = L2f[x96, chunk] rhs = MT -> (chunk, x')
  All PSUM evacuation copies are FD=768 (2 PSUM banks) and are split across
  DVE and ACT by a weighted round-robin (both engines read PSUM at 1 f32/cyc;
  this is the critical resource).  Output staged [128, 12, 192] bf16 and
  DMA'd with 4608B-contiguous per-partition runs (>=512B avoids the SDMA
  read-modify-write penalty that made 384B-run writes ~2x slower).  C-groups
  of task t are partly held back and ride along the next phase so the
  out-DMA stream never starves.  Compute bf16 (PSUM fp32); output written
  bf16, upcast on host (rel err ~5.3e-3 vs the reference).
"""

import math
import os
import sys

import numpy as np

for _p in ("/opt/trn_rl_repo",):
    if _p not in sys.path and os.path.isdir(_p):
        sys.path.insert(0, _p)

import ml_dtypes  # noqa: E402

BF16 = ml_dtypes.bfloat16

POLE = math.sqrt(3.0) - 2.0
GAIN = (1.0 - POLE) * (1.0 - 1.0 / POLE)  # 6.0
N = 96
F = 2
NOUT = N * F  # 192
NCORES = 8
ZSLICE = 48

# per core: 3 tasks as (volume_index, zslice_index); t0,t1 share slot0's
# volume, t2 uses slot1's (cores with one distinct volume duplicate it)
TASKS = [
    [(0, 0), (0, 1), (0, 2)],
    [(1, 0), (1, 1), (0, 3)],
    [(1, 2), (1, 3), (2, 0)],
    [(2, 1), (2, 2), (2, 3)],
    [(3, 0), (3, 1), (3, 2)],
    [(4, 0), (4, 1), (3, 3)],
    [(4, 2), (4, 3), (5, 0)],
    [(5, 1), (5, 2), (5, 3)],
]


def _cubic(t):
    a = np.abs(t)
    out = (2.0 / 3.0 + (0.5 * a - 1.0) * a**2) * (a < 1)
    out = out + (-((a - 2.0) ** 3) / 6.0) * ((a >= 1) & (a < 2))
    return out


def _prefilter_mat(n):
    """96x96 matrix of the causal+anticausal cubic-spline prefilter (float64)."""
    p = POLE
    xm = np.eye(n, dtype=np.float64) * GAIN
    i = np.arange(n)
    pows = p**i + p ** (2 * n - 1 - i)
    c = np.zeros((n, n), dtype=np.float64)
    c[0] = (pows @ xm) * (p / (1.0 - p ** (2 * n))) + xm[0]
    for k in range(1, n):
        c[k] = xm[k] + p * c[k - 1]
    out = np.zeros((n, n), dtype=np.float64)
    out[n - 1] = c[n - 1] * (p / (p - 1.0))
    for k in range(n - 2, -1, -1):
        out[k] = p * (out[k + 1] - c[k])
    return out


def _upsample_mat(n, f=F):
    """2n x n matrix of the edge-padded stride-2 transposed conv (float64)."""
    k = 4 * f  # f even -> is_odd == 0
    start = 1.0 / (2 * f) - 2.0
    pts = np.arange(k, dtype=np.float64) * (1.0 / f) + start
    ker = _cubic(pts)
    npad = n + 4
    U = np.zeros((f * n, npad), dtype=np.float64)
    for o in range(f * n):
        for i in range(npad):
            s = o + (k - 1) - f * i
            if 0 <= s < k:
                U[o, i] += ker[s]
    Uc = np.zeros((f * n, n), dtype=np.float64)
    for i in range(npad):
        j = min(max(i - 2, 0), n - 1)
        Uc[:, j] += U[:, i]
    return Uc


def build_M():
    """Exact 192x96 per-axis operator (float64)."""
    return _upsample_mat(N) @ _prefilter_mat(N)


_NC_CACHE = {}


def _strip_redundant_self_waits(nc):
    """Drop sem waits that are trivially satisfied by same-engine program order.

    Tile's per-proc wait emission is not transitively minimal: a PE matmul can
    end up waiting on the PE's own semaphore (already guaranteed by in-order
    engine execution) in addition to a cross-engine wait, and the MM ISA
    struct only has one sync-wait slot (walrus: "Too many sync wait
    commands"). A wait on sem S is redundant for instruction I on engine E iff
    S is only ever updated by E and the cumulative updates to S from E before
    I already reach the wait value.
    """
    import concourse.mybir as mybir

    for fn in nc.m.functions:
        for blk in fn.blocks:
            updaters = {}  # sem id -> set of engines updating it (block-wide)
            for i in blk.instructions:
                si = i.sync_info
                if si is None:
                    continue
                for u in si.on_update or []:
                    updaters.setdefault(u.id, set()).add(i.engine)
            seen = {}  # (engine, sem id) -> cumulative update count so far
            for i in blk.instructions:
                si = i.sync_info
                if si is None:
                    continue
                if si.on_wait:
                    kept = []
                    for w in si.on_wait:
                        if (
                            w.sync_type == "semaphore"
                            and w.wait_mode == "sem-ge-imm"
                            and updaters.get(w.id) == {i.engine}
                            and seen.get((i.engine, w.id), 0) >= w.wait_value
                        ):
                            continue  # implied by program order
                        kept.append(w)
                    if len(kept) != len(si.on_wait):
                        si.on_wait[:] = kept
                for u in si.on_update or []:
                    key = (i.engine, u.id)
                    seen[key] = seen.get(key, 0) + u.update_value
            # each engine ISA struct has a single sync-wait slot: offload
            # extra waits onto same-engine nops inserted just before
            new_insts = []
            nop_n = 0
            for i in blk.instructions:
                si = i.sync_info
                if si is not None and si.on_wait and len(si.on_wait) > 1:
                    extra = list(si.on_wait[:-1])
                    si.on_wait[:] = [si.on_wait[-1]]
                    for w in extra:
                        nop = mybir.InstNoOp(
                            name=f"I-waitnop-{blk.name}-{nop_n}", ins=[], outs=[]
                        )
                        nop_n += 1
                        nop.engine = i.engine
                        nop.sync_info = mybir.SyncInfo(on_wait=[w], on_update=[])
                        new_insts.append(nop)
                new_insts.append(i)
            if nop_n:
                blk.instructions[:] = new_insts


def _hoist_input_dmas(nc, n_hoist=3):
    """Move the first input DMAs ahead of the sync engine's entry barrier.

    The Tile/BSP prologue (entry EVSEM barrier + TENSOR_LOAD) delays the
    first dma_start by several us. The leading input DMAs have no waits
    (inputs are resident at NEFF start, dst tiles untouched), so issuing
    them first starts the HBM reads during the prologue.
    """
    import concourse.mybir as mybir

    blocks = nc.m.functions[0].blocks
    body = blocks[1]
    dmas = []
    for i in body.instructions:
        if type(i).__name__ == "InstDMACopy" and i.engine == mybir.EngineType.SP:
            si = i.sync_info
            if si is not None and si.on_wait:
                break  # stop at the first gated DMA
            dmas.append(i)
            if len(dmas) >= n_hoist:
                break
    if not dmas:
        return
    dset = set(id(x) for x in dmas)
    body.instructions[:] = [i for i in body.instructions if id(i) not in dset]
    # insert into the prologue block after the leading InstCall, ahead of
    # the entry barrier: the sync engine starts immediately, so these DMAs
    # issue at t~0 while the other engines are still loading their code
    pro = blocks[0].instructions
    pos = 1 if pro and type(pro[0]).__name__ == "InstCall" else 0
    pro[:] = pro[:pos] + dmas + pro[pos:]


def build_nc():
    import concourse.bass as bass
    import concourse.mybir as mybir
    from concourse.tile import TileContext

    bf16 = mybir.dt.bfloat16
    f32 = mybir.dt.float32

    nc = bass.Bass(enable_partition_id=False)
    # inputs: 2 volume slots (z unpadded: 96 rows; y padded to 128 per
    # x-slice for 128-col FWL stationaries), spline matrices packed in one
    # [96, 336] tile: [0:96] = MzT slices for t0|t1, [96:144] = MzT for t2,
    # [144:336] = MT
    vol_ext = nc.declare_dram_parameter("vol", [2, 96, 96 * 128], bf16, isOutput=False)
    mts_ext = nc.declare_dram_parameter("mts", [96, 336], bf16, isOutput=False)
    # out rows: task t, stage-group d (12 chunks of 128 (z'y')-rows each),
    # partition-major inside: per partition 2304 elems contiguous
    out_ext = nc.declare_dram_parameter("out", [128, 3, 6, 2304], bf16, isOutput=True)
    dbg = os.environ.get("KDBG") == "1"
    if dbg:
        dbg_l1 = nc.declare_dram_parameter("dbg_l1", [96, 3, 48 * 96], bf16, isOutput=True)
        dbg_l2 = nc.declare_dram_parameter("dbg_l2", [96, 3, 9216], bf16, isOutput=True)

    with TileContext(nc) as tc:
        with (
            tc.tile_pool(name="consts", bufs=1) as consts,
            tc.tile_pool(name="vols", bufs=2) as vols_pool,
            tc.tile_pool(name="l1", bufs=2) as l1_pool,
            tc.tile_pool(name="l2", bufs=3) as l2_pool,
            tc.tile_pool(name="stage", bufs=8) as stage_pool,
            tc.tile_pool(name="pab", bufs=2, space="PSUM") as pab_pool,
            tc.tile_pool(name="pc", bufs=4, space="PSUM") as pc_pool,
        ):
            mts = consts.tile([96, 336], bf16)
            nc.sync.dma_start(out=mts[:], in_=mts_ext[:])
            # prime the scalar engine's activation table during the input
            # DMA wait (ACT_TABLE_LOAD ~1.3us otherwise lands on the first
            # real C-copy); the dummy also touches nothing downstream
            prime = consts.tile([96, 136], bf16)
            nc.gpsimd.memset(prime[:], 0.0)
            nc.scalar.copy(prime[0:1, 4:8], prime[0:1, 0:4])

            # graduated input chunks: small early ones so stage A starts as
            # soon as the first x-slices land
            def load_vol(slot, bounds):
                vol = vols_pool.tile([96, 96 * 128], bf16, name="vol")
                for ch in range(len(bounds) - 1):
                    nc.sync.dma_start(
                        out=vol[:, bounds[ch] : bounds[ch + 1]],
                        in_=vol_ext[slot, :, bounds[ch] : bounds[ch + 1]],
                    )
                return vol

            vol0 = load_vol(0, [0, 1024, 2048, 4096, 6656, 9472, 12288])
            vol1 = load_vol(1, [0, 4096, 8192, 12288])

            # ---- PSUM evacuation engine assignment ----
            # both engines read PSUM at 1 f32/cycle.  strict per-stream
            # alternation: consecutive generations of one stream go to
            # different engines so each stream gets 2-wide copy concurrency
            # (different PSUM banks, so DVE+ACT run in parallel).
            tick = {"n": 0}

            def copy_psum(stream, dst, src):
                # global strict alternation: emission order approximates
                # readiness order, so alternating engines per copy keeps both
                # PSUM read ports busy with no same-engine back-to-back pairs
                t = tick["n"]
                tick["n"] = t + 1
                if t % 2 == 0:
                    nc.vector.tensor_copy(dst, src)
                else:
                    nc.scalar.copy(dst, src)

            # ---- stage C emitter per task ----
            # 72 chunks of 128 (z'y')-rows; groups of 4 chunks (one 2-bank
            # PSUM tile); 3 groups fill one [128, 12, 192] stage tile ->
            # one out-DMA with 4608B contiguous per-partition runs.
            def make_emit_c(t, L2f):
                state = {"stage": None}

                def emit_c_group(cg):
                    if cg % 6 == 0:
                        state["stage"] = stage_pool.tile([128, 12, 192], bf16, name="stage")
                    stage = state["stage"]
                    pc = pc_pool.tile(
                        [128, 2, 192], f32, name="pc", tag="pc", padded_shape=[128, 2, 256]
                    )
                    for j in range(2):
                        ch = cg * 2 + j
                        nc.tensor.matmul(
                            pc[:, j, :],
                            lhsT=L2f[:, ch * 128 : (ch + 1) * 128],
                            rhs=mts[:, 144:336],
                            start=True,
                            stop=True,
                        )
                    copy_psum("c", stage[:, (cg % 6) * 2 : (cg % 6) * 2 + 2, :], pc[:, :, :])
                    if t == 2 and cg == 33:
                        # fine tail: ship the last stage group in two halves
                        nc.sync.dma_start(
                            out=out_ext[:, t, 5, 0:1536],
                            in_=stage[:, 0:8, :].rearrange("p c x -> p (c x)"),
                        )
                    elif t == 2 and cg == 35:
                        nc.sync.dma_start(
                            out=out_ext[:, t, 5, 1536:2304],
                            in_=stage[:, 8:12, :].rearrange("p c x -> p (c x)"),
                        )
                    elif cg % 6 == 5:
                        d = cg // 6
                        nc.sync.dma_start(
                            out=out_ext[:, t, d, :],
                            in_=stage[:].rearrange("p c x -> p (c x)"),
                        )

                return emit_c_group

            # ---- stage A ----
            # slot0: fused t0+t1 (N=96); slot1: t2 alone (N=48).
            # pa partitions = y (96 real + 32 junk from the y-pad cols); only
            # [0:96] is copied out, so the junk never propagates.
            def emit_a0(vol, L1a):
                for g in range(12):
                    if g % 2 == 0:  # 8 x-slices -> one 2-bank pab tile
                        pa = pab_pool.tile(
                            [128, 8, 96], f32, name="pa0", tag="pab",
                            padded_shape=[128, 8, 128],
                        )
                        for j in range(8):
                            x = g * 8 + j
                            nc.tensor.matmul(
                                pa[:, j, :],
                                lhsT=vol[:, x * 128 : (x + 1) * 128],
                                rhs=mts[:, 0:96],
                                start=True,
                                stop=True,
                            )
                        copy_psum(
                            "a0",
                            L1a[:, :, :, g * 8 : (g + 1) * 8],
                            pa[0:96, :, :].rearrange("p j (t z) -> p t z j", t=2),
                        )
                    else:  # two 4-x-slice 1-bank pc-pool tiles
                        for h in range(2):
                            pa = pc_pool.tile(
                                [128, 4, 96], f32, name="pa0h", tag="pc",
                                padded_shape=[128, 4, 128],
                            )
                            for j in range(4):
                                x = g * 8 + h * 4 + j
                                nc.tensor.matmul(
                                    pa[:, j, :],
                                    lhsT=vol[:, x * 128 : (x + 1) * 128],
                                    rhs=mts[:, 0:96],
                                    start=True,
                                    stop=True,
                                )
                            copy_psum(
                                "a0",
                                L1a[:, :, :, g * 8 + h * 4 : g * 8 + h * 4 + 4],
                                pa[0:96, :, :].rearrange("p j (t z) -> p t z j", t=2),
                            )

            def make_a1_thunks(vol, L1b):
                def mk(g):
                    def thunk():
                        pa = pab_pool.tile(
                            [128, 16, 48], f32, name="pa1", tag="pab",
                            padded_shape=[128, 16, 64],
                        )
                        for j in range(16):
                            x = g * 16 + j
                            nc.tensor.matmul(
                                pa[:, j, :],
                                lhsT=vol[:, x * 128 : (x + 1) * 128],
                                rhs=mts[:, 96:144],
                                start=True,
                                stop=True,
                            )
                        copy_psum(
                            "a1",
                            L1b[:, :, g * 16 : (g + 1) * 16],
                            pa[0:96, :, :].rearrange("p j z -> p z j"),
                        )
                    return thunk

                return [mk(g) for g in range(6)]

            # ---- unified B/C scheduler ----
            # After A0, the three tasks' B-streams round-robin (B2 joins once
            # slot1's stage A finishes) and each task's C-groups are emitted
            # as soon as their L2 rows are safely ahead.  One continuous
            # phase: no pipeline-refill bubbles at task boundaries, PE never
            # idles long enough for HAM to re-throttle, and the copy engines
            # always have 2+ independent streams to alternate across.
            def emit_a1_group(vol, L1b, g):
                pa = pab_pool.tile(
                    [128, 16, 48], f32, name="pa1", tag="pab",
                    padded_shape=[128, 16, 64],
                )
                for j in range(16):
                    x = g * 16 + j
                    nc.tensor.matmul(
                        pa[:, j, :],
                        lhsT=vol[:, x * 128 : (x + 1) * 128],
                        rhs=mts[:, 96:144],
                        start=True,
                        stop=True,
                    )
                copy_psum(
                    "a1",
                    L1b[:, :, g * 16 : (g + 1) * 16],
                    pa[0:96, :, :].rearrange("p j z -> p z j"),
                )

            class BC:
                def __init__(self, t, l1_slice):
                    self.t = t
                    self.l1_slice = l1_slice
                    self.L2 = l2_pool.tile([96, 48, 192], bf16, name="l2")
                    self.L2f = self.L2[:].rearrange("p a b -> p (a b)")
                    self.emit_c = make_emit_c(t, self.L2f)
                    self.b_done = 0
                    self.c_done = 0

                def step_b(self):
                    pb = pab_pool.tile(
                        [128, 4, 192], f32, name="pb", tag="pab",
                        padded_shape=[128, 4, 256],
                    )
                    for jj in range(4):
                        zp = self.b_done * 4 + jj
                        nc.tensor.matmul(
                            pb[0:96, jj, :],
                            lhsT=self.l1_slice(zp),
                            rhs=mts[:, 144:336],
                            start=True,
                            stop=True,
                        )
                    copy_psum(
                        "b",
                        self.L2[:, self.b_done * 4 : self.b_done * 4 + 4, :],
                        pb[0:96, :, :],
                    )
                    self.b_done += 1

                def pump_c(self, max_groups=4):
                    # one-B-group slack while B is in flight: a C-group at
                    # the exact readiness boundary would head-of-line block
                    # the PE queue waiting on the just-emitted B copy
                    rows = self.b_done * 768 - (768 if self.b_done < 12 else 0)
                    n = 0
                    while (
                        self.c_done < 36
                        and n < max_groups
                        and (self.c_done + 1) * 256 <= rows
                    ):
                        self.emit_c(self.c_done)
                        self.c_done += 1
                        n += 1

            # ---- schedule ----
            L1a = l1_pool.tile([96, 2, 48, 96], bf16, name="l1a")
            emit_a0(vol0, L1a)
            if dbg:
                nc.sync.dma_start(out=dbg_l1[:, 0:2, :], in_=L1a[:].rearrange("p t z x -> p t (z x)"))

            L1b = l1_pool.tile([96, 48, 96], bf16, name="l1b")
            streams = [
                BC(0, lambda zp: L1a[:, 0, zp, :]),
                BC(1, lambda zp: L1a[:, 1, zp, :]),
            ]
            s2 = BC(2, lambda zp: L1b[:, zp, :])
            a1_next = 0
            total_b = 0
            rr = 0
            while True:
                alive = [s for s in streams if s.b_done < 12]
                if not alive and a1_next >= 6:
                    break
                if alive:
                    s = alive[rr % len(alive)]
                    rr += 1
                    s.step_b()
                    total_b += 1
                if a1_next < 6 and total_b % 2 == 0:
                    emit_a1_group(vol1, L1b, a1_next)
                    a1_next += 1
                    if a1_next == 6:
                        streams.append(s2)
                for s in streams:
                    s.pump_c(4)
            if dbg:
                nc.sync.dma_start(out=dbg_l1[:, 2, :], in_=L1b[:].rearrange("p z x -> p (z x)"))
            while any(s.c_done < 36 for s in streams):
                for s in streams:
                    s.pump_c(6)
            if dbg:
                for s in streams:
                    nc.sync.dma_start(out=dbg_l2[:, s.t, :], in_=s.L2f)

    if os.environ.get("KNOPASS") != "1":
        _strip_redundant_self_waits(nc)
        _hoist_input_dmas(nc)
    return nc


def make_in_maps(volume, M):
    MT = np.ascontiguousarray(M.T).astype(BF16)  # (96, 192)
    in_maps = []
    for core in range(NCORES):
        tasks = TASKS[core]
        slot_vols = [tasks[0][0], tasks[2][0]]
        vols = np.zeros((2, 96, 96, 128), dtype=BF16)
        for s, v in enumerate(slot_vols):
            b, c = divmod(v, 3)
            vt = np.transpose(volume[b, c], (0, 2, 1))  # (z, x, y)
            vols[s, :, :, :96] = vt.astype(BF16)
        mts = np.zeros((96, 336), dtype=BF16)
        for t in range(2):
            sl = tasks[t][1]
            mts[:, t * 48 : (t + 1) * 48] = MT[:, sl * ZSLICE : (sl + 1) * ZSLICE]
        sl = tasks[2][1]
        mts[:, 96:144] = MT[:, sl * ZSLICE : (sl + 1) * ZSLICE]
        mts[:, 144:336] = MT
        in_maps.append({"vol": vols.reshape(2, 96, 96 * 128), "mts": mts})
    return in_maps


def gather_out(results):
    out = np.zeros((2, 3, 192, 192, 192), dtype=np.float32)
    for core in range(NCORES):
        o = np.asarray(results[core]["out"], dtype=np.float32)
        # [128, 3, 6, 2304] -> (t, d, cc, p, x) -> task rows (z'y') x x'
        o = o.reshape(128, 3, 6, 12, 192).transpose(1, 2, 3, 0, 4)
        o = o.reshape(3, 9216, 192).reshape(3, 48, 192, 192)
        for t in range(3):
            v, s = TASKS[core][t]
            b, c = divmod(v, 3)
            out[b, c, s * ZSLICE : (s + 1) * ZSLICE] = o[t]
    return out


def run(volume, trace=False):
    """Returns (output, exec_time_ns_or_None)."""
    import concourse.bass_utils as bu
    from concourse.bass_utils import run_bass_kernel_spmd

    if trace:
        # avoid the S3 artifact upload in the axon trace path
        bu.upload_artifacts = lambda tmpdir: str(tmpdir)

    volume = np.asarray(volume, dtype=np.float32)
    M = build_M()
    in_maps = make_in_maps(volume, M)
    if "nc" not in _NC_CACHE:
        _NC_CACHE["nc"] = build_nc()
    nc = _NC_CACHE["nc"]
    res = run_bass_kernel_spmd(
        nc, in_maps, core_ids=list(range(NCORES)), trace=trace
    )
    out = gather_out(res.results)
    return out, getattr(res, "exec_time_ns", None)


def kernel(volume):
    out, _ = run(volume, trace=False)
    return out


# revision 18
# speedup vs baseline: 1.1148x; 1.0347x over previous
"""Trainium2 Bass kernel: cubic B-spline upsampling x2 of a (2,3,96,96,96) volume.

Math: the reference op (recursive IIR prefilter along each spatial axis, then
an 8-tap stride-2 transposed conv along each axis) is linear and separable.
The whole per-axis operator is a dense 192x96 matrix M (built exactly on the
host in float64).  out = M (x) M (x) M applied along z, y, x.

Device strategy (8 NeuronCores, SPMD, no collectives), v2:
  24 slices = 6 (b,c) volumes x 4 z'-slices of 48 rows; 3 per core, arranged
  so tasks t0,t1 share one volume (slot0) and t2 uses a second (slot1) --
  input DMA is 2 volume loads per core instead of 3, z unpadded (96 rows).
  Three data-stationary matmul stages (stationary = data tile, moving =
  spline matrix), so no transposes are needed:
    A: per 8-x group: lhsT = vol[z96, (x,y128)]  rhs=MzT-slices -> (y, z')
       slot0 runs t0+t1 fused (N=96, one LDWEIGHTS pass over the volume)
    B: per z': lhsT = L1[y96, x96]   rhs = MT (96,192) -> (x, y')
    C: per 128-chunk of (z'y'): lhsT = L2f[x96, chunk] rhs = MT -> (chunk, x')
  All PSUM evacuation copies are FD=768 (2 PSUM banks) and are split across
  DVE and ACT by a weighted round-robin (both engines read PSUM at 1 f32/cyc;
  this is the critical resource).  Output staged [128, 12, 192] bf16 and
  DMA'd with 4608B-contiguous per-partition runs (>=512B avoids the SDMA
  read-modify-write penalty that made 384B-run writes ~2x slower).  C-groups
  of task t are partly held back and ride along the next phase so the
  out-DMA stream never starves.  Compute bf16 (PSUM fp32); output written
  bf16, upcast on host (rel err ~5.3e-3 vs the reference).
"""

import math
import os
import sys

import numpy as np

for _p in ("/opt/trn_rl_repo",):
    if _p not in sys.path and os.path.isdir(_p):
        sys.path.insert(0, _p)

import ml_dtypes  # noqa: E402

BF16 = ml_dtypes.bfloat16

POLE = math.sqrt(3.0) - 2.0
GAIN = (1.0 - POLE) * (1.0 - 1.0 / POLE)  # 6.0
N = 96
F = 2
NOUT = N * F  # 192
NCORES = 8
ZSLICE = 48

# per core: 3 tasks as (volume_index, zslice_index); t0,t1 share slot0's
# volume, t2 uses slot1's (cores with one distinct volume duplicate it)
TASKS = [
    [(0, 0), (0, 1), (0, 2)],
    [(1, 0), (1, 1), (0, 3)],
    [(1, 2), (1, 3), (2, 0)],
    [(2, 1), (2, 2), (2, 3)],
    [(3, 0), (3, 1), (3, 2)],
    [(4, 0), (4, 1), (3, 3)],
    [(4, 2), (4, 3), (5, 0)],
    [(5, 1), (5, 2), (5, 3)],
]


def _cubic(t):
    a = np.abs(t)
    out = (2.0 / 3.0 + (0.5 * a - 1.0) * a**2) * (a < 1)
    out = out + (-((a - 2.0) ** 3) / 6.0) * ((a >= 1) & (a < 2))
    return out


def _prefilter_mat(n):
    """96x96 matrix of the causal+anticausal cubic-spline prefilter (float64)."""
    p = POLE
    xm = np.eye(n, dtype=np.float64) * GAIN
    i = np.arange(n)
    pows = p**i + p ** (2 * n - 1 - i)
    c = np.zeros((n, n), dtype=np.float64)
    c[0] = (pows @ xm) * (p / (1.0 - p ** (2 * n))) + xm[0]
    for k in range(1, n):
        c[k] = xm[k] + p * c[k - 1]
    out = np.zeros((n, n), dtype=np.float64)
    out[n - 1] = c[n - 1] * (p / (p - 1.0))
    for k in range(n - 2, -1, -1):
        out[k] = p * (out[k + 1] - c[k])
    return out


def _upsample_mat(n, f=F):
    """2n x n matrix of the edge-padded stride-2 transposed conv (float64)."""
    k = 4 * f  # f even -> is_odd == 0
    start = 1.0 / (2 * f) - 2.0
    pts = np.arange(k, dtype=np.float64) * (1.0 / f) + start
    ker = _cubic(pts)
    npad = n + 4
    U = np.zeros((f * n, npad), dtype=np.float64)
    for o in range(f * n):
        for i in range(npad):
            s = o + (k - 1) - f * i
            if 0 <= s < k:
                U[o, i] += ker[s]
    Uc = np.zeros((f * n, n), dtype=np.float64)
    for i in range(npad):
        j = min(max(i - 2, 0), n - 1)
        Uc[:, j] += U[:, i]
    return Uc


def build_M():
    """Exact 192x96 per-axis operator (float64)."""
    return _upsample_mat(N) @ _prefilter_mat(N)


_NC_CACHE = {}


def _strip_redundant_self_waits(nc):
    """Drop sem waits that are trivially satisfied by same-engine program order.

    Tile's per-proc wait emission is not transitively minimal: a PE matmul can
    end up waiting on the PE's own semaphore (already guaranteed by in-order
    engine execution) in addition to a cross-engine wait, and the MM ISA
    struct only has one sync-wait slot (walrus: "Too many sync wait
    commands"). A wait on sem S is redundant for instruction I on engine E iff
    S is only ever updated by E and the cumulative updates to S from E before
    I already reach the wait value.
    """
    import concourse.mybir as mybir

    for fn in nc.m.functions:
        for blk in fn.blocks:
            updaters = {}  # sem id -> set of engines updating it (block-wide)
            for i in blk.instructions:
                si = i.sync_info
                if si is None:
                    continue
                for u in si.on_update or []:
                    updaters.setdefault(u.id, set()).add(i.engine)
            seen = {}  # (engine, sem id) -> cumulative update count so far
            for i in blk.instructions:
                si = i.sync_info
                if si is None:
                    continue
                if si.on_wait:
                    kept = []
                    for w in si.on_wait:
                        if (
                            w.sync_type == "semaphore"
                            and w.wait_mode == "sem-ge-imm"
                            and updaters.get(w.id) == {i.engine}
                            and seen.get((i.engine, w.id), 0) >= w.wait_value
                        ):
                            continue  # implied by program order
                        kept.append(w)
                    if len(kept) != len(si.on_wait):
                        si.on_wait[:] = kept
                for u in si.on_update or []:
                    key = (i.engine, u.id)
                    seen[key] = seen.get(key, 0) + u.update_value
            # each engine ISA struct has a single sync-wait slot: offload
            # extra waits onto same-engine nops inserted just before
            new_insts = []
            nop_n = 0
            for i in blk.instructions:
                si = i.sync_info
                if si is not None and si.on_wait and len(si.on_wait) > 1:
                    extra = list(si.on_wait[:-1])
                    si.on_wait[:] = [si.on_wait[-1]]
                    for w in extra:
                        nop = mybir.InstNoOp(
                            name=f"I-waitnop-{blk.name}-{nop_n}", ins=[], outs=[]
                        )
                        nop_n += 1
                        nop.engine = i.engine
                        nop.sync_info = mybir.SyncInfo(on_wait=[w], on_update=[])
                        new_insts.append(nop)
                new_insts.append(i)
            if nop_n:
                blk.instructions[:] = new_insts


def _hoist_input_dmas(nc, n_hoist=3):
    """Move the first input DMAs ahead of the sync engine's entry barrier.

    The Tile/BSP prologue (entry EVSEM barrier + TENSOR_LOAD) delays the
    first dma_start by several us. The leading input DMAs have no waits
    (inputs are resident at NEFF start, dst tiles untouched), so issuing
    them first starts the HBM reads during the prologue.
    """
    import concourse.mybir as mybir

    blocks = nc.m.functions[0].blocks
    body = blocks[1]
    dmas = []
    for i in body.instructions:
        if type(i).__name__ == "InstDMACopy" and i.engine == mybir.EngineType.SP:
            si = i.sync_info
            if si is not None and si.on_wait:
                break  # stop at the first gated DMA
            dmas.append(i)
            if len(dmas) >= n_hoist:
                break
    if not dmas:
        return
    dset = set(id(x) for x in dmas)
    body.instructions[:] = [i for i in body.instructions if id(i) not in dset]
    # insert into the prologue block after the leading InstCall, ahead of
    # the entry barrier: the sync engine starts immediately, so these DMAs
    # issue at t~0 while the other engines are still loading their code
    pro = blocks[0].instructions
    pos = 1 if pro and type(pro[0]).__name__ == "InstCall" else 0
    pro[:] = pro[:pos] + dmas + pro[pos:]


def build_nc():
    import concourse.bass as bass
    import concourse.mybir as mybir
    from concourse.tile import TileContext

    bf16 = mybir.dt.bfloat16
    f32 = mybir.dt.float32

    nc = bass.Bass(enable_partition_id=False)
    # inputs: 2 volume slots (z unpadded: 96 rows; y padded to 128 per
    # x-slice for 128-col FWL stationaries), spline matrices packed in one
    # [96, 336] tile: [0:96] = MzT slices for t0|t1, [96:144] = MzT for t2,
    # [144:336] = MT
    vol_ext = nc.declare_dram_parameter("vol", [2, 96, 96 * 128], bf16, isOutput=False)
    mts_ext = nc.declare_dram_parameter("mts", [96, 336], bf16, isOutput=False)
    # out rows: task t, stage-group d (12 chunks of 128 (z'y')-rows each),
    # partition-major inside: per partition 2304 elems contiguous
    out_ext = nc.declare_dram_parameter("out", [128, 3, 6, 2304], bf16, isOutput=True)
    dbg = os.environ.get("KDBG") == "1"
    if dbg:
        dbg_l1 = nc.declare_dram_parameter("dbg_l1", [96, 3, 48 * 96], bf16, isOutput=True)
        dbg_l2 = nc.declare_dram_parameter("dbg_l2", [96, 3, 9216], bf16, isOutput=True)

    with TileContext(nc) as tc:
        with (
            tc.tile_pool(name="consts", bufs=1) as consts,
            tc.tile_pool(name="vols", bufs=2) as vols_pool,
            tc.tile_pool(name="l1", bufs=2) as l1_pool,
            tc.tile_pool(name="l2", bufs=3) as l2_pool,
            tc.tile_pool(name="stage", bufs=8) as stage_pool,
            tc.tile_pool(name="pab", bufs=2, space="PSUM") as pab_pool,
            tc.tile_pool(name="pc", bufs=4, space="PSUM") as pc_pool,
        ):
            mts = consts.tile([96, 336], bf16)
            nc.sync.dma_start(out=mts[:], in_=mts_ext[:])
            # prime the scalar engine's activation table during the input
            # DMA wait (ACT_TABLE_LOAD ~1.3us otherwise lands on the first
            # real C-copy); the dummy also touches nothing downstream
            prime = consts.tile([96, 136], bf16)
            nc.gpsimd.memset(prime[:], 0.0)
            nc.scalar.copy(prime[0:1, 4:8], prime[0:1, 0:4])

            # graduated input chunks: small early ones so stage A starts as
            # soon as the first x-slices land
            def load_vol(slot, bounds):
                vol = vols_pool.tile([96, 96 * 128], bf16, name="vol")
                for ch in range(len(bounds) - 1):
                    nc.sync.dma_start(
                        out=vol[:, bounds[ch] : bounds[ch + 1]],
                        in_=vol_ext[slot, :, bounds[ch] : bounds[ch + 1]],
                    )
                return vol

            vol0 = load_vol(0, [0, 1024, 2048, 4096, 6656, 9472, 12288])
            vol1 = load_vol(1, [0, 4096, 8192, 12288])

            # ---- PSUM evacuation engine assignment ----
            # both engines read PSUM at 1 f32/cycle.  strict per-stream
            # alternation: consecutive generations of one stream go to
            # different engines so each stream gets 2-wide copy concurrency
            # (different PSUM banks, so DVE+ACT run in parallel).
            est = {"v": 0.0, "s": 0.0}

            def copy_psum(stream, dst, src):
                # cost-weighted balance: DVE is relatively cheaper on small
                # copies, ACT on large ones; keep accumulated busy equal so
                # neither PSUM read port becomes the single bottleneck
                fd = 1
                for n in src.shape[1:]:
                    fd *= n
                cv = (120 + fd) / 0.96 + 30.0
                cs = (172 + fd) / 1.2 + 140.0
                if est["v"] + cv <= est["s"] + cs:
                    est["v"] += cv
                    nc.vector.tensor_copy(dst, src)
                else:
                    est["s"] += cs
                    nc.scalar.copy(dst, src)

            # ---- stage C emitter per task ----
            # 72 chunks of 128 (z'y')-rows; groups of 4 chunks (one 2-bank
            # PSUM tile); 3 groups fill one [128, 12, 192] stage tile ->
            # one out-DMA with 4608B contiguous per-partition runs.
            def make_emit_c(t, L2f):
                state = {"stage": None}

                def emit_c_group(cg):
                    if cg % 6 == 0:
                        state["stage"] = stage_pool.tile([128, 12, 192], bf16, name="stage")
                    stage = state["stage"]
                    pc = pc_pool.tile(
                        [128, 2, 192], f32, name="pc", tag="pc", padded_shape=[128, 2, 256]
                    )
                    for j in range(2):
                        ch = cg * 2 + j
                        nc.tensor.matmul(
                            pc[:, j, :],
                            lhsT=L2f[:, ch * 128 : (ch + 1) * 128],
                            rhs=mts[:, 144:336],
                            start=True,
                            stop=True,
                        )
                    copy_psum("c", stage[:, (cg % 6) * 2 : (cg % 6) * 2 + 2, :], pc[:, :, :])
                    if t == 2 and cg == 33:
                        # fine tail: ship the last stage group in two halves
                        nc.sync.dma_start(
                            out=out_ext[:, t, 5, 0:1536],
                            in_=stage[:, 0:8, :].rearrange("p c x -> p (c x)"),
                        )
                    elif t == 2 and cg == 35:
                        nc.sync.dma_start(
                            out=out_ext[:, t, 5, 1536:2304],
                            in_=stage[:, 8:12, :].rearrange("p c x -> p (c x)"),
                        )
                    elif cg % 6 == 5:
                        d = cg // 6
                        nc.sync.dma_start(
                            out=out_ext[:, t, d, :],
                            in_=stage[:].rearrange("p c x -> p (c x)"),
                        )

                return emit_c_group

            # ---- stage A ----
            # slot0: fused t0+t1 (N=96); slot1: t2 alone (N=48).
            # pa partitions = y (96 real + 32 junk from the y-pad cols); only
            # [0:96] is copied out, so the junk never propagates.
            def emit_a0(vol, L1a):
                for g in range(12):
                    if g % 2 == 0:  # 8 x-slices -> one 2-bank pab tile
                        pa = pab_pool.tile(
                            [128, 8, 96], f32, name="pa0", tag="pab",
                            padded_shape=[128, 8, 128],
                        )
                        for j in range(8):
                            x = g * 8 + j
                            nc.tensor.matmul(
                                pa[:, j, :],
                                lhsT=vol[:, x * 128 : (x + 1) * 128],
                                rhs=mts[:, 0:96],
                                start=True,
                                stop=True,
                            )
                        copy_psum(
                            "a0",
                            L1a[:, :, :, g * 8 : (g + 1) * 8],
                            pa[0:96, :, :].rearrange("p j (t z) -> p t z j", t=2),
                        )
                    else:  # two 4-x-slice 1-bank pc-pool tiles
                        for h in range(2):
                            pa = pc_pool.tile(
                                [128, 4, 96], f32, name="pa0h", tag="pc",
                                padded_shape=[128, 4, 128],
                            )
                            for j in range(4):
                                x = g * 8 + h * 4 + j
                                nc.tensor.matmul(
                                    pa[:, j, :],
                                    lhsT=vol[:, x * 128 : (x + 1) * 128],
                                    rhs=mts[:, 0:96],
                                    start=True,
                                    stop=True,
                                )
                            copy_psum(
                                "a0",
                                L1a[:, :, :, g * 8 + h * 4 : g * 8 + h * 4 + 4],
                                pa[0:96, :, :].rearrange("p j (t z) -> p t z j", t=2),
                            )

            def make_a1_thunks(vol, L1b):
                def mk(g):
                    def thunk():
                        pa = pab_pool.tile(
                            [128, 16, 48], f32, name="pa1", tag="pab",
                            padded_shape=[128, 16, 64],
                        )
                        for j in range(16):
                            x = g * 16 + j
                            nc.tensor.matmul(
                                pa[:, j, :],
                                lhsT=vol[:, x * 128 : (x + 1) * 128],
                                rhs=mts[:, 96:144],
                                start=True,
                                stop=True,
                            )
                        copy_psum(
                            "a1",
                            L1b[:, :, g * 16 : (g + 1) * 16],
                            pa[0:96, :, :].rearrange("p j z -> p z j"),
                        )
                    return thunk

                return [mk(g) for g in range(6)]

            # ---- unified B/C scheduler ----
            # After A0, the three tasks' B-streams round-robin (B2 joins once
            # slot1's stage A finishes) and each task's C-groups are emitted
            # as soon as their L2 rows are safely ahead.  One continuous
            # phase: no pipeline-refill bubbles at task boundaries, PE never
            # idles long enough for HAM to re-throttle, and the copy engines
            # always have 2+ independent streams to alternate across.
            def emit_a1_group(vol, L1b, g):
                pa = pab_pool.tile(
                    [128, 16, 48], f32, name="pa1", tag="pab",
                    padded_shape=[128, 16, 64],
                )
                for j in range(16):
                    x = g * 16 + j
                    nc.tensor.matmul(
                        pa[:, j, :],
                        lhsT=vol[:, x * 128 : (x + 1) * 128],
                        rhs=mts[:, 96:144],
                        start=True,
                        stop=True,
                    )
                copy_psum(
                    "a1",
                    L1b[:, :, g * 16 : (g + 1) * 16],
                    pa[0:96, :, :].rearrange("p j z -> p z j"),
                )

            class BC:
                def __init__(self, t, l1_slice):
                    self.t = t
                    self.l1_slice = l1_slice
                    self.L2 = l2_pool.tile([96, 48, 192], bf16, name="l2")
                    self.L2f = self.L2[:].rearrange("p a b -> p (a b)")
                    self.emit_c = make_emit_c(t, self.L2f)
                    self.b_done = 0
                    self.c_done = 0

                def step_b(self):
                    pb = pab_pool.tile(
                        [128, 4, 192], f32, name="pb", tag="pab",
                        padded_shape=[128, 4, 256],
                    )
                    for jj in range(4):
                        zp = self.b_done * 4 + jj
                        nc.tensor.matmul(
                            pb[0:96, jj, :],
                            lhsT=self.l1_slice(zp),
                            rhs=mts[:, 144:336],
                            start=True,
                            stop=True,
                        )
                    copy_psum(
                        "b",
                        self.L2[:, self.b_done * 4 : self.b_done * 4 + 4, :],
                        pb[0:96, :, :],
                    )
                    self.b_done += 1

                def pump_c(self, max_groups=4):
                    # one-B-group slack while B is in flight: a C-group at
                    # the exact readiness boundary would head-of-line block
                    # the PE queue waiting on the just-emitted B copy
                    rows = self.b_done * 768 - (768 if self.b_done < 12 else 0)
                    n = 0
                    while (
                        self.c_done < 36
                        and n < max_groups
                        and (self.c_done + 1) * 256 <= rows
                    ):
                        self.emit_c(self.c_done)
                        self.c_done += 1
                        n += 1

            # ---- schedule ----
            L1a = l1_pool.tile([96, 2, 48, 96], bf16, name="l1a")
            emit_a0(vol0, L1a)
            if dbg:
                nc.sync.dma_start(out=dbg_l1[:, 0:2, :], in_=L1a[:].rearrange("p t z x -> p t (z x)"))

            L1b = l1_pool.tile([96, 48, 96], bf16, name="l1b")
            streams = [
                BC(0, lambda zp: L1a[:, 0, zp, :]),
                BC(1, lambda zp: L1a[:, 1, zp, :]),
            ]
            s2 = BC(2, lambda zp: L1b[:, zp, :])
            a1_next = 0
            total_b = 0
            rr = 0
            while True:
                alive = [s for s in streams if s.b_done < 12]
                if not alive and a1_next >= 6:
                    break
                if alive:
                    s = alive[rr % len(alive)]
                    rr += 1
                    s.step_b()
                    total_b += 1
                if a1_next < 6 and total_b % 2 == 0:
                    emit_a1_group(vol1, L1b, a1_next)
                    a1_next += 1
                    if a1_next == 6:
                        streams.append(s2)
                for s in streams:
                    s.pump_c(4)
            if dbg:
                nc.sync.dma_start(out=dbg_l1[:, 2, :], in_=L1b[:].rearrange("p z x -> p (z x)"))
            while any(s.c_done < 36 for s in streams):
                for s in streams:
                    s.pump_c(6)
            if dbg:
                for s in streams:
                    nc.sync.dma_start(out=dbg_l2[:, s.t, :], in_=s.L2f)

    if os.environ.get("KNOPASS") != "1":
        _strip_redundant_self_waits(nc)
        _hoist_input_dmas(nc)
    return nc


def make_in_maps(volume, M):
    MT = np.ascontiguousarray(M.T).astype(BF16)  # (96, 192)
    in_maps = []
    for core in range(NCORES):
        tasks = TASKS[core]
        slot_vols = [tasks[0][0], tasks[2][0]]
        vols = np.zeros((2, 96, 96, 128), dtype=BF16)
        for s, v in enumerate(slot_vols):
            b, c = divmod(v, 3)
            vt = np.transpose(volume[b, c], (0, 2, 1))  # (z, x, y)
            vols[s, :, :, :96] = vt.astype(BF16)
        mts = np.zeros((96, 336), dtype=BF16)
        for t in range(2):
            sl = tasks[t][1]
            mts[:, t * 48 : (t + 1) * 48] = MT[:, sl * ZSLICE : (sl + 1) * ZSLICE]
        sl = tasks[2][1]
        mts[:, 96:144] = MT[:, sl * ZSLICE : (sl + 1) * ZSLICE]
        mts[:, 144:336] = MT
        in_maps.append({"vol": vols.reshape(2, 96, 96 * 128), "mts": mts})
    return in_maps


def gather_out(results):
    out = np.zeros((2, 3, 192, 192, 192), dtype=np.float32)
    for core in range(NCORES):
        o = np.asarray(results[core]["out"], dtype=np.float32)
        # [128, 3, 6, 2304] -> (t, d, cc, p, x) -> task rows (z'y') x x'
        o = o.reshape(128, 3, 6, 12, 192).transpose(1, 2, 3, 0, 4)
        o = o.reshape(3, 9216, 192).reshape(3, 48, 192, 192)
        for t in range(3):
            v, s = TASKS[core][t]
            b, c = divmod(v, 3)
            out[b, c, s * ZSLICE : (s + 1) * ZSLICE] = o[t]
    return out


def run(volume, trace=False):
    """Returns (output, exec_time_ns_or_None)."""
    import concourse.bass_utils as bu
    from concourse.bass_utils import run_bass_kernel_spmd

    if trace:
        # avoid the S3 artifact upload in the axon trace path
        bu.upload_artifacts = lambda tmpdir: str(tmpdir)

    volume = np.asarray(volume, dtype=np.float32)
    M = build_M()
    in_maps = make_in_maps(volume, M)
    if "nc" not in _NC_CACHE:
        _NC_CACHE["nc"] = build_nc()
    nc = _NC_CACHE["nc"]
    res = run_bass_kernel_spmd(
        nc, in_maps, core_ids=list(range(NCORES)), trace=trace
    )
    out = gather_out(res.results)
    return out, getattr(res, "exec_time_ns", None)


def kernel(volume):
    out, _ = run(volume, trace=False)
    return out
